# revision 1
# baseline (speedup 1.0000x reference)
"""Trainium2 Bass kernel for CGL contrastive region loss.

Problem: proj (96, 256, 64, 64) f32 = 3 stacked views of B=32 images.
Only views 2 and 3 (aug1/aug2) are used. From each image, 25 regions
(5x5 grid of 2x2 windows at centres {10..50}) are extracted over all 256
channels -> region vectors of D = 256*2*2 = 1024. Per image pair the
loss reduces to: for each row r of the 50x50 Gram matrix G of the
stacked normalized regions [u1;u2] (scaled by 1/TAU), LSE over the full
row excluding only the main diagonal entry, minus the positive logit
pos_r = S[r, (r+25)%50]. Data-parallel over batch (4 pairs/core, 8
cores), scalar partials summed on the host.

Device pipeline per core (all 4 pairs batched in 50x200 tiles):
  bf16 inputs, 4 gram chains (8 matmuls each, K=128) into ONE 50x200
  PSUM tile.  diag: one identity mul + one blocked reduce -> d [50,4].
  inv = sqrt(10)*rsqrt(d) = exp(-0.5*ln(d/10)) on ACT.
  G+mask -> SBUF (mask = -1e30 on each block diagonal: exact row-LSE
  diagonal removal; doubles as the PSUM->SBUF move).
  colscale broadcast binv4 = ones^T @ diag(inv) (one f32r PE matmul),
  S = (G+mask) * binv4 * inv_row (two DVE muls), eall = exp(S - 10)
  (one ACT op, valid LSE shift since diag(S)=10), esum = blocked row
  reduce.  lse-10 = ln(esum) (ACT).  pos: -2*pos via a -2*I25 constant
  mul + reduce, summed together with ln(esum) in one 50x8 reduce.
  total = partition-sum matmul; scale + (+10 shift restored) constant
  folded into one tensor_scalar; result leaves via reg_load + posted
  TENSOR_STORE (no output DMA ring round-trip).

ACT tables: every activation (Ln, Exp) is served by the single function
set `natural_log_exp_and_others`, forced by pointing both bacc's
insert_act_table_loads and walrus (BASS_ACT_ROOT_JSON_PATH) at a
patched act_info.json in which no other set contains exp/ln. One table
load, pulled to the head of the ACT queue by a dummy activation and
hidden under the input DMAs. (The default greedy assignment alternates
natural_log/exp_and_others sets, reloading a 1.3us table at every
Ln<->Exp transition, several on the critical path.)

Span overheads trimmed: Bass-init const memsets + entry all-engine
barrier deleted from the BIR (the NRT preamble already runs two
all-engine rendezvous and no const APs are referenced), so the input
DMA triggers issue right after the NRT preamble. Tile tail uses a
sem-only drain barrier. The NRT preamble (~5.5-7us) and postamble
semaphore wipe (~6.5us) are runtime-injected and immovable.
"""

import os
import numpy as np

NB = 4                    # pairs per core
NCORES = 8
R = 25
FREE = NB * 8 * 50        # 1600 free elements per core
_CENTRES = (10, 20, 30, 40, 50)

# cf layout (f32r bits, mostly consumed through an f32 bitcast view;
# per-pair block constants are free-dim stride-0 broadcasts of one copy):
#   [0:50)    ident: 50x50 identity
#   [50:100)  mask: -1e30 on the diagonal, 0 elsewhere
#   [100:150) ones 50x50 (f32r stationary for the colscale matmul;
#             col 100 doubles as the ones column for the final sum)
#   [150:175) negident25: rows 0:25 = -2 * I25 (positive-logit
#             extraction, the -2 loss weight pre-folded)
#   [175] -10.0   [176] 0.0
_CF_COLS = 177

_nc_cache = None


def _build_consts():
    cf = np.zeros((50, _CF_COLS), dtype=np.float32)
    cf[:, 0:50] = np.eye(50)
    cf[:, 50:100] = np.eye(50) * np.float32(-1e30)
    cf[:, 100:150] = 1.0
    cf[0:25, 150:175] = np.eye(25) * np.float32(-2.0)
    cf[:, 175] = -10.0
    cf[:, 176] = 0.0
    return cf


def _patched_act_root():
    """Stage a copy of the neuronxcc pwp table dir whose act_info.json
    leaves `natural_log_exp_and_others` as the only set containing exp or
    ln, so every activation resolves to one table set (single load)."""
    import json
    import shutil
    import tempfile

    import neuronxcc

    src = os.path.join(os.path.dirname(neuronxcc.__file__), "pwp", "pwp_bin_trainium")
    dst = os.path.join(tempfile.gettempdir(), "pwp_nlexp_%d" % os.getuid())
    marker = os.path.join(dst, ".patched_ok")
    if not os.path.exists(marker):
        if os.path.exists(dst):
            shutil.rmtree(dst)
        shutil.copytree(src, dst)
        p = os.path.join(dst, "act_info.json")
        os.chmod(p, 0o644)
        with open(p) as f:
            d = json.load(f)
        for e in d["act_func_sets"]:
            if e["name"] != "natural_log_exp_and_others":
                e["act"].pop("exp", None)
                e["act"].pop("ln", None)
        with open(p, "w") as f:
            json.dump(d, f)
        with open(marker, "w") as f:
            f.write("ok")
    return os.path.join(dst, "act_info.json")


def _apply_act_surgery():
    import functools
    import json

    import concourse.bacc as baccmod

    act_json = _patched_act_root()
    os.environ["BASS_ACT_ROOT_JSON_PATH"] = act_json

    @functools.cache
    def patched_tables(arch):
        from concourse import mybir

        with open(act_json) as f:
            d = json.load(f)
        return {
            e["name"]: {
                mybir.ActivationFunctionType.from_pwp(v) for v in e["act"].keys()
            }
            for e in d["act_func_sets"]
        }

    baccmod.get_activation_tables = patched_tables


def _strip_init_overhead(nc):
    """Remove the Bass-init const memsets and entry all-engine barrier from
    the 'main' block. No const APs are referenced by this kernel, and the
    NRT preamble already synchronizes all engines before the program runs."""
    from concourse import mybir

    for func in nc.m.functions:
        for blk in func.blocks:
            if blk.name != "main":
                continue
            kept = []
            for inst in blk.instructions:
                if isinstance(
                    inst,
                    (mybir.InstMemset, mybir.InstDrain, mybir.InstEventSemaphore),
                ):
                    continue
                kept.append(inst)
            blk.instructions[:] = kept


def _build_nc():
    _apply_act_surgery()

    import concourse.bacc as bacc
    import concourse.tile as tile
    from concourse import mybir
    from concourse.vector_clock import ScopedClock

    class FastTailTileContext(tile.TileContext):
        """Tile tail without the two full all-engine barriers.

        The sync-engine drain already waits on the global vector clock
        (every instruction's sem tick), so once it completes nothing is
        in flight; a sem-only EVSEM barrier then orders the gpsimd
        sem_clears after it."""

        def _drain_and_barrier(self, tick_clock, wait_clock):
            drain_inst = self.nc.sync.drain()
            wait_clock.add_sem_waits(
                drain_inst.ins, ScopedClock({None: tick_clock.global_clock})
            )
            self.nc.all_engine_barrier(sem_only=True)
            popped = self.nc._tile_sem_poison_stack.pop()
            assert popped is self._sem_poison
            self.nc.clear_and_free_semaphores(list(self.sems.allocated().values()))

    f32 = mybir.dt.float32
    f32r = mybir.dt.float32r
    bf16 = mybir.dt.bfloat16
    i32 = mybir.dt.int32
    Alu = mybir.AluOpType
    Act = mybir.ActivationFunctionType
    X = mybir.AxisListType.X

    nc = bacc.Bacc("TRN2", target_bir_lowering=False, debug=False)
    u_dram = nc.dram_tensor("u", [128, FREE], bf16, kind="ExternalInput").ap()
    cf_dram = nc.dram_tensor("cf", [50, _CF_COLS], f32r, kind="ExternalInput").ap()
    out_h = nc.dram_tensor("out", [1, 1], f32, kind="ExternalOutput")
    out_dram = out_h.ap()
    # runtime-populated pointer to the output buffer: loading it into a
    # register pair EARLY (under the DMA window) keeps the ~1us DRAM
    # pointer fetch off the critical path of the final store
    out_ptr = nc.pointer_tensor(out_h)

    def blk(ap, f=50):
        return ap.rearrange("p (b f) -> p b f", f=f)

    # raw (non-tile) SBUF scalar for the final result so the reg_load
    # sees a concrete (non-symbolic) access pattern
    res_t = nc.alloc_sbuf_tensor("res_scalar", [1, 1], f32)

    import contextlib

    _regs = contextlib.ExitStack()
    opair = _regs.enter_context(nc.gpsimd.register64("outaddr"))
    rreg = _regs.enter_context(nc.gpsimd.register("resreg"))
    # pre-TileContext 'main'-block instruction: the gpsimd queue executes it
    # immediately after the NRT preamble, hiding the ~1us DRAM pointer
    # fetch under the input DMA window (the tile scheduler would otherwise
    # sink it next to the final store). u64 pointer = 2 x i32 registers.
    nc.gpsimd.load(opair, out_ptr.ap().bitcast(mybir.dt.int32))

    with FastTailTileContext(nc) as tc:
        with (
            tc.tile_pool(name="data", bufs=1) as data,
            tc.tile_pool(name="consts", bufs=1) as consts,
            tc.tile_pool(name="work", bufs=2) as work,
            tc.tile_pool(name="psg", bufs=1, space="PSUM") as psg,
            tc.tile_pool(name="psb", bufs=1, space="PSUM") as psb,
            tc.tile_pool(name="pst", bufs=1, space="PSUM") as pst,
        ):
            H = FREE // 2
            # u halves (1600B rows keep the DMA rings at full burst
            # efficiency): pairs 0-1 on the sync HWDGE ring, 2-3 on
            # scalar; the small const tensor follows on the sync ring
            # (a partition-split across both rings was tried and is a
            # wash: 64-partition transfers halve per-ring throughput)
            ubs = []
            cf = consts.tile([50, _CF_COLS], f32r)
            for h in range(2):
                ubh = data.tile([128, H], bf16, tag=f"ub{h}")
                eng = nc.sync if h == 0 else nc.scalar
                eng.dma_start(ubh[:], u_dram[:, h * H : (h + 1) * H])
                ubs.append(ubh)
            nc.sync.dma_start(cf[:], cf_dram)

            cff = cf[:].bitcast(f32)
            ident = cff[:, 0:50]
            mask = cff[:, 50:100]
            ones50r = cf[:, 100:150]
            ones_col = cff[:, 100:101]
            negident = cff[0:25, 150:175]
            b_m10 = cff[:, 175:176]
            b_zero = cff[:, 176:177]
            identB = ident.unsqueeze(1).broadcast_to([50, NB, 50])
            maskB = mask.unsqueeze(1).broadcast_to([50, NB, 50])
            negidentB = negident.unsqueeze(1).broadcast_to([25, NB, 25])

            # dummy activation on a memset scratch (no DMA deps): pulls the
            # single ACT table load to the head of the ACT queue, fully
            # hidden under the input DMAs
            tscr = work.tile([1, 1], f32, tag="tscr")
            nc.vector.memset(tscr[:], 1.0)
            nc.scalar.activation(tscr[:], tscr[:], Act.Exp, bias=tscr[:])

            # PE p-state warmup: sustained dummy matmuls during the input
            # DMA window so the real gram chains run at full clock
            wscr = work.tile([128, 400], bf16, tag="wscr")
            nc.vector.memset(wscr[:], 0.0)
            psw = pst.tile([50, 400], f32, tag="warm")
            for w in range(4):
                nc.tensor.matmul(
                    psw[:], wscr[:, 0:50], wscr[:],
                    start=(w == 0), stop=(w == 3),
                )

            # zero rows 25:50 of the -2*pos half of the final-sum tile
            fin = work.tile([50, 2 * NB], f32, tag="fin")
            nc.vector.memset(fin[:], 0.0)

            # 4 gram chains into one 50x200 PSUM tile
            gpall = psg.tile([50, 200], f32, tag="g")
            for b in range(NB):
                gslice = gpall[:, b * 50 : (b + 1) * 50]
                base = (b % 2) * 400
                for k in range(8):
                    sl = ubs[b // 2][:, base + k * 50 : base + (k + 1) * 50]
                    nc.tensor.matmul(gslice, sl, sl, start=(k == 0), stop=(k == 7))

            # block diagonals -> squared norms d [50,4]
            dmul = work.tile([50, 200], f32, tag="dmul")
            nc.vector.tensor_mul(blk(dmul[:]), blk(gpall[:]), identB)
            dsq = work.tile([50, NB], f32, tag="dsq")
            nc.vector.reduce_sum(dsq[:], blk(dmul[:]), axis=X)

            # inv = sqrt(10)*rsqrt(d) = exp(-0.5*ln(d/10)) on ACT
            lnd = work.tile([50, NB], f32, tag="lnd")
            nc.scalar.activation(lnd[:], dsq[:], Act.Ln, bias=b_zero, scale=0.1)
            inv = work.tile([50, NB], f32, tag="inv")
            nc.scalar.activation(inv[:], lnd[:], Act.Exp, bias=b_zero, scale=-0.5)

            # PSUM -> SBUF move doubling as the exact diagonal kill
            gsb = work.tile([50, 200], f32, tag="gsb")
            nc.vector.tensor_add(blk(gsb[:]), blk(gpall[:]), maskB)

            # colscale row: binv4[p,(b,c)] = inv[c,b] via ones^T @ diag(inv)
            invrep = inv[:].unsqueeze(2).broadcast_to([50, NB, 50])
            dinv = work.tile([50, 200], f32r, tag="dinv")
            nc.vector.tensor_mul(blk(dinv[:]), identB, invrep)
            binv4 = psb.tile([50, 200], f32, tag="binv4")
            nc.tensor.matmul(binv4[:], ones50r, dinv[:], start=True, stop=True)

            # S = (G+mask) * colscale * rowscale; eall = exp(S-10)
            gcol = work.tile([50, 200], f32, tag="gcol")
            nc.vector.tensor_mul(gcol[:], gsb[:], binv4[:])
            rowsc = work.tile([50, 200], f32, tag="rowsc")
            nc.vector.tensor_mul(blk(rowsc[:]), blk(gcol[:]), invrep)
            eall = work.tile([50, 200], f32, tag="eall")
            nc.scalar.activation(eall[:], rowsc[:], Act.Exp, bias=b_m10)
            esum = work.tile([50, NB], f32, tag="esum")
            nc.vector.reduce_sum(esum[:], blk(eall[:]), axis=X)

            # -2 * positives from the S12/S21 block diagonals of S
            pmul = work.tile([25, NB * 25], f32, tag="pmul")
            nc.vector.tensor_mul(
                blk(pmul[:], f=25),
                blk(rowsc[0:25, :])[:, :, 25:50],
                negidentB,
            )
            nc.vector.reduce_sum(fin[0:25, NB : 2 * NB], blk(pmul[:], f=25), axis=X)

            # lse-10 = ln(esum); one 50x8 reduce sums lse and -2*pos rows
            nc.scalar.activation(fin[:, 0:NB], esum[:], Act.Ln, bias=b_zero)
            acc = work.tile([50, 1], f32, tag="acc")
            nc.vector.reduce_sum(acc[:], fin[:], axis=X)

            # partition sum -> raw scalar (the +10 LSE shift constant and
            # the 1/(2*R*B) scaling are applied on the host, per the
            # data-parallel contract); gpsimd cannot read PSUM, so one DVE
            # copy bounces it to SBUF
            tp = pst.tile([1, 1], f32, tag="tot")
            nc.tensor.matmul(tp[:], acc[:], ones_col, start=True, stop=True)
            nc.vector.tensor_copy(res_t.ap(), tp[:])

            # posted TENSOR_STORE of the scalar result through the
            # preloaded address pair: no output DMA round-trip, no pointer
            # fetch on the critical path; the store flight is covered by
            # the tile drain barrier. (registers are untyped 32-bit —
            # move the f32 bits as i32)
            nc.gpsimd.reg_load(rreg, res_t.ap().bitcast(i32))
            nc.gpsimd.store(opair, rreg)

    _regs.close()

    _strip_init_overhead(nc)
    nc.compile()
    return nc


def get_nc():
    global _nc_cache
    if _nc_cache is None:
        _nc_cache = _build_nc()
    return _nc_cache


def pack_inputs(proj: np.ndarray) -> np.ndarray:
    """(96,256,64,64) -> (128, 32, 8, 50) bf16: partition=c%128,
    free=(pair, chunk k=(cb,dy,dx), view, region rh*5+rw)."""
    import ml_dtypes

    win = np.array([[c - 1, c] for c in _CENTRES])  # (5, 2): rows/cols of each window
    v = np.stack([proj[32:64], proj[64:96]], axis=1)  # (32, 2, 256, 64, 64)
    g = v[:, :, :, win[:, :, None, None], win[None, None, :, :]]  # (32,2,256,5,2,5,2)
    g = g.reshape(32, 2, 2, 128, 5, 2, 5, 2)  # b, view, cb, c', rh, dy, rw, dx
    arr = np.transpose(g, (3, 0, 2, 5, 7, 1, 4, 6))  # c', b, cb, dy, dx, view, rh, rw
    return np.ascontiguousarray(arr).reshape(128, 32, 8, 50).astype(ml_dtypes.bfloat16)


def kernel(proj: np.ndarray) -> np.ndarray:
    from concourse.bass_utils import run_bass_kernel_spmd

    nc = get_nc()
    arr = pack_inputs(np.asarray(proj))
    cf = _build_consts()
    in_maps = [
        {
            "u": np.ascontiguousarray(arr[:, c * NB : (c + 1) * NB]).reshape(128, FREE),
            "cf": cf,
        }
        for c in range(NCORES)
    ]
    results = run_bass_kernel_spmd(nc, in_maps, list(range(NCORES))).results
    # device partials are raw sums of (lse-10) and -2*pos rows: restore the
    # +10 LSE shift (10*50*NB per core) and divide by 2*R*B_global
    total = 0.0
    for r in results:
        total += (float(r["out"][0, 0]) + 10.0 * 50 * NB) / (2.0 * R * NB * NCORES)
    return np.float32(total)



# revision 10
# speedup vs baseline: 1.1687x; 1.1687x over previous
"""Trainium2 Bass kernel for CGL contrastive region loss.

Problem: proj (96, 256, 64, 64) f32 = 3 stacked views of B=32 images.
Only views 2 and 3 (aug1/aug2) are used. From each image, 25 regions
(5x5 grid of 2x2 windows at centres {10..50}) are extracted over all 256
channels -> region vectors of D = 256*2*2 = 1024. Per image pair the
loss reduces to: for each row r of the 50x50 Gram matrix G of the
stacked normalized regions [u1;u2] (scaled by 1/TAU), LSE over the full
row excluding only the main diagonal entry, minus the positive logit
pos_r = S[r, (r+25)%50]. Data-parallel over batch (4 pairs/core, 8
cores).

Device pipeline per core (v2):
  Host L2-normalizes each region vector and folds in sqrt(1/TAU), so the
  Gram IS the logit matrix S directly (diag exactly ~10). Input ships as
  fp8e4 [128, 1600]: free = (group:2, ko:4, ki:2, col:100), two pairs
  stacked per 100-col group.  One DMA on the sync ring.
  PE: per group, a bf16 [103,100]x[103,100] "mask matmul" seeds PSUM with
  M = -C*delta - C + C*sameblock (kills the main diagonal and the
  cross-pair garbage blocks; C=200), then 4 fp8 DoubleRow matmuls
  (K=256 each) accumulate the Gram on top -> PSUM holds S+M [100, 200].
  ACT: per group, exp(S+M-10) with accum_out -> row sums esum [100,2]
  directly (eall scratch write is dead).  DVE (parallel): -2*pos via a
  two-slice affine-select constant mul + blocked reduce -> [100,2].
  One 1600B output DMA ships fin=[esum|-2pos] [100,4]; host does
  ln(esum), the +10 LSE shift, and the global scale/sum.

ACT tables: only Exp is needed on device (ln runs on the host), served
by `exp_and_others`, forced single-set by pointing both bacc's
insert_act_table_loads and walrus (BASS_ACT_ROOT_JSON_PATH) at a patched
act_info.json in which no other set contains exp. The table load is
pulled to the head of the ACT queue by a dummy activation, hidden under
the input DMA.

Span overheads trimmed: Bass-init const memsets + entry all-engine
barrier deleted from the BIR (the NRT preamble already runs two
all-engine rendezvous and no const APs are referenced). Tile tail uses
a sem-only drain barrier. The NRT preamble (~7us to program start) and
postamble semaphore wipe (~7us) are runtime-injected and immovable.
"""

import os
import numpy as np

NB = 4                    # pairs per core
NCORES = 8
R = 25
_CENTRES = (10, 20, 30, 40, 50)
SQC = np.float32(np.sqrt(10.0))   # sqrt(1/TAU)
MASK_S = 200.0 ** 0.5             # sqrt(C): mask magnitude C=200

_nc_cache = None


def _patched_act_root():
    """Stage a copy of the neuronxcc pwp table dir whose act_info.json
    leaves `exp_and_others` as the only set containing exp, so the single
    activation function used on device resolves to one table set."""
    import json
    import shutil
    import tempfile

    import neuronxcc

    src = os.path.join(os.path.dirname(neuronxcc.__file__), "pwp", "pwp_bin_trainium")
    dst = os.path.join(tempfile.gettempdir(), "pwp_exponly_%d" % os.getuid())
    marker = os.path.join(dst, ".patched_ok")
    if not os.path.exists(marker):
        if os.path.exists(dst):
            shutil.rmtree(dst)
        shutil.copytree(src, dst)
        p = os.path.join(dst, "act_info.json")
        os.chmod(p, 0o644)
        with open(p) as f:
            d = json.load(f)
        for e in d["act_func_sets"]:
            if e["name"] != "exp_and_others":
                e["act"].pop("exp", None)
        with open(p, "w") as f:
            json.dump(d, f)
        with open(marker, "w") as f:
            f.write("ok")
    return os.path.join(dst, "act_info.json")


def _apply_act_surgery():
    import functools
    import json

    import concourse.bacc as baccmod

    act_json = _patched_act_root()
    os.environ["BASS_ACT_ROOT_JSON_PATH"] = act_json

    @functools.cache
    def patched_tables(arch):
        from concourse import mybir

        with open(act_json) as f:
            d = json.load(f)
        return {
            e["name"]: {
                mybir.ActivationFunctionType.from_pwp(v) for v in e["act"].keys()
            }
            for e in d["act_func_sets"]
        }

    baccmod.get_activation_tables = patched_tables


def _strip_init_overhead(nc):
    """Remove the Bass-init const memsets and entry all-engine barrier from
    the 'main' block. No const APs are referenced by this kernel, and the
    NRT preamble already synchronizes all engines before the program runs."""
    from concourse import mybir

    for func in nc.m.functions:
        for blk in func.blocks:
            if blk.name != "main":
                continue
            kept = []
            for inst in blk.instructions:
                if isinstance(
                    inst,
                    (mybir.InstMemset, mybir.InstDrain, mybir.InstEventSemaphore),
                ):
                    continue
                kept.append(inst)
            blk.instructions[:] = kept


def _build_nc():
    _apply_act_surgery()

    import concourse.bacc as bacc
    import concourse.tile as tile
    from concourse import mybir
    from concourse.vector_clock import ScopedClock

    class FastTailTileContext(tile.TileContext):
        """Tile tail without the two full all-engine barriers.

        The sync-engine drain already waits on the global vector clock
        (every instruction's sem tick), so once it completes nothing is
        in flight; a sem-only EVSEM barrier then orders the gpsimd
        sem_clears after it."""

        def _drain_and_barrier(self, tick_clock, wait_clock):
            drain_inst = self.nc.sync.drain()
            wait_clock.add_sem_waits(
                drain_inst.ins, ScopedClock({None: tick_clock.global_clock})
            )
            self.nc.all_engine_barrier(sem_only=True)
            popped = self.nc._tile_sem_poison_stack.pop()
            assert popped is self._sem_poison
            self.nc.clear_and_free_semaphores(list(self.sems.allocated().values()))

    f32 = mybir.dt.float32
    bf16 = mybir.dt.bfloat16
    fp8 = mybir.dt.float8e4
    Alu = mybir.AluOpType
    Act = mybir.ActivationFunctionType
    X = mybir.AxisListType.X
    DR = mybir.MatmulPerfMode.DoubleRow

    nc = bacc.Bacc("TRN2", target_bir_lowering=False, debug=False)
    u_dram = nc.dram_tensor("u", [128, 1792], fp8, kind="ExternalInput").ap()
    out_dram = nc.dram_tensor("out", [100, 4], f32, kind="ExternalOutput").ap()

    with FastTailTileContext(nc) as tc:
        with (
            tc.tile_pool(name="data", bufs=1) as data,
            tc.tile_pool(name="consts", bufs=1) as consts,
            tc.tile_pool(name="work", bufs=2) as work,
            tc.tile_pool(name="psg", bufs=1, space="PSUM") as psg,
            tc.tile_pool(name="pst", bufs=1, space="PSUM") as pst,
        ):
            # input DMA first: one [128, 1600] fp8 transfer on the sync ring
            ub = data.tile([128, 1792], fp8, tag="ub")
            nc.sync.dma_start(ub[:], u_dram)

            # ---- on-device constants (synthesized during the DMA window) ----
            # All compute-engine APs must start at partition 0, so the mask
            # matmul operands live in three aligned tiles -> 3 tiny matmuls:
            #   diag [100,100]: stat -s / mov +s on the diagonal -> -C*delta
            #   crow [1,100]:   stat -s / mov +s everywhere      -> -C
            #   brow [2,100]:   both +s on 50-block indicators   -> +C*same
            dstat = consts.tile([100, 100], bf16, tag="dstat")
            dmov = consts.tile([100, 100], bf16, tag="dmov")
            nc.vector.memset(dstat[:], 0.0)
            nc.vector.memset(dmov[:], 0.0)
            # (affine_select runs on gpsimd; idle during the DMA window)
            nc.gpsimd.affine_select(
                dstat[:], dstat[:],
                pattern=[[1, 100]], compare_op=Alu.not_equal,
                fill=-MASK_S, base=0, channel_multiplier=-1,
            )
            nc.gpsimd.affine_select(
                dmov[:], dmov[:],
                pattern=[[1, 100]], compare_op=Alu.not_equal,
                fill=MASK_S, base=0, channel_multiplier=-1,
            )
            cstat = consts.tile([1, 100], bf16, tag="cstat")
            cmov = consts.tile([1, 100], bf16, tag="cmov")
            nc.vector.memset(cstat[:], -MASK_S)
            nc.vector.memset(cmov[:], MASK_S)
            # brow row p covers cols [50p, 50p+50)
            brow = consts.tile([2, 100], bf16, tag="brow")
            nc.vector.memset(brow[:], MASK_S)
            nc.gpsimd.affine_select(
                brow[:], brow[:],
                pattern=[[1, 100]], compare_op=Alu.is_ge,
                fill=0.0, base=0, channel_multiplier=-50,
            )
            nc.gpsimd.affine_select(
                brow[:], brow[:],
                pattern=[[-1, 100]], compare_op=Alu.is_gt,
                fill=0.0, base=50, channel_multiplier=50,
            )

            # negident: -2 at (r, 25+r) for r in [0,25) and (50+r, 75+r).
            # op1 marks the full f-p==25 stripe (also hits rows 25..49 at
            # f in [50,75), which land on garbage cross-blocks); op2 zeroes
            # the f in [50,75) band, which contains no wanted entries.
            negid = consts.tile([100, 100], f32, tag="negid")
            nc.vector.memset(negid[:], 0.0)
            nc.gpsimd.affine_select(
                negid[:], negid[:],
                pattern=[[1, 100]], compare_op=Alu.not_equal,
                fill=-2.0, base=-25, channel_multiplier=-1,
            )
            nc.gpsimd.affine_select(
                negid[:], negid[:],
                pattern=[[-25, 2], [1, 50]], compare_op=Alu.is_ge,
                fill=0.0, base=0, channel_multiplier=0,
            )
            negidB = negid[:].unsqueeze(1).broadcast_to([100, 2, 100])

            # bias column for exp(S - 10)
            b_m10 = consts.tile([100, 1], f32, tag="bm10")
            nc.vector.memset(b_m10[:], -10.0)

            # dummy activation on a memset scratch (no DMA deps): pulls the
            # single ACT table load to the head of the ACT queue, fully
            # hidden under the input DMA
            tscr = work.tile([1, 1], f32, tag="tscr")
            nc.vector.memset(tscr[:], 1.0)
            nc.scalar.activation(tscr[:], tscr[:], Act.Exp, bias=tscr[:])

            # PE p-state warmup: sustained dummy matmuls during the input
            # DMA window so the real gram chains run at full clock
            wscr = work.tile([128, 400], bf16, tag="wscr")
            nc.vector.memset(wscr[:], 0.0)
            psw = pst.tile([50, 400], f32, tag="warm")
            for w in range(4):
                nc.tensor.matmul(
                    psw[:], wscr[:, 0:50], wscr[:],
                    start=(w == 0), stop=(w == 3),
                )

            # ---- gram + mask: PSUM [100, 200] = S + M for 2 groups ----
            gp = psg.tile([100, 200], f32, tag="g")
            for g in range(2):
                gs = gp[:, g * 100 : (g + 1) * 100]
                # mask matmuls seed PSUM (consts only: run under the DMA)
                nc.tensor.matmul(gs, dstat[:], dmov[:], start=True, stop=False,
                                 skip_group_check=True)
                nc.tensor.matmul(gs, cstat[:], cmov[:], start=False, stop=False,
                                 skip_group_check=True)
                nc.tensor.matmul(gs, brow[:], brow[:], start=False, stop=False,
                                 skip_group_check=True)
                for ko in range(4):
                    # col dim padded 100->112: DoubleRow needs the k-tile
                    # stride 16B-aligned; only cols 0:100 are read
                    sl = ub[:, g * 896 + ko * 224 : g * 896 + (ko + 1) * 224]
                    sl = sl.rearrange("p (ki c) -> p ki c", ki=2)[:, :, 0:100]
                    nc.tensor.matmul(gs, sl, sl, start=False, stop=(ko == 3),
                                     perf_mode=DR, skip_group_check=True)

            # fin = [esum(2) | -2*possum(2)]
            fin = work.tile([100, 4], f32, tag="fin")

            # exp(S + M - 10) with fused row-sum -> esum per group
            eall = work.tile([100, 200], bf16, tag="eall")
            for g in range(2):
                nc.scalar.activation(
                    eall[:, g * 100 : (g + 1) * 100],
                    gp[:, g * 100 : (g + 1) * 100],
                    Act.Exp, bias=b_m10,
                    accum_out=fin[:, g : g + 1],
                )

            # -2 * positives from the S12 block diagonals (parallel on DVE)
            pmul = work.tile([100, 200], f32, tag="pmul")
            pblk = pmul[:].rearrange("p (b f) -> p b f", f=100)
            nc.vector.tensor_mul(pblk, gp[:].rearrange("p (b f) -> p b f", f=100), negidB)
            nc.vector.reduce_sum(fin[:, 2:4], pblk, axis=X)

            # ship [esum | -2pos]; ln + shift + scaling happen on the host
            nc.sync.dma_start(out_dram, fin[:])

    _strip_init_overhead(nc)
    nc.compile()
    return nc


def get_nc():
    global _nc_cache
    if _nc_cache is None:
        _nc_cache = _build_nc()
    return _nc_cache


def pack_inputs(proj: np.ndarray) -> np.ndarray:
    """(96,256,64,64) -> (8, 128, 1600) fp8e4: per core, partition=feature
    p (f = (ko*2+ki)*128 + p), free=(group, ko, ki, pairin*50 + view*25+reg).
    Region vectors are L2-normalized and scaled by sqrt(1/TAU) on the host,
    so the device Gram is the logit matrix directly."""
    import ml_dtypes

    win = np.array([[c - 1, c] for c in _CENTRES])  # (5, 2)
    v = np.stack([proj[32:64], proj[64:96]], axis=1)  # (32, 2, 256, 64, 64)
    g = v[:, :, :, win[:, :, None, None], win[None, None, :, :]]  # (32,2,256,5,2,5,2)
    # region vector = flatten (C, dy, dx); reorder to (b, view, rh, rw, C, dy, dx)
    g = np.transpose(g, (0, 1, 3, 5, 2, 4, 6)).reshape(32, 2, 25, 1024)
    nrm = np.sqrt(np.sum(g.astype(np.float32) ** 2, axis=-1, keepdims=True))
    g = g / np.maximum(nrm, 1e-12) * SQC  # (32, 2, 25, 1024)
    # stack views: col50 = view*25 + reg
    g = g.reshape(32, 50, 1024)
    # feature f -> (ko, ki, p)
    g = g.reshape(32, 50, 4, 2, 128)
    # per core: [pair(4), col50, ko, ki, p] -> [p, group, ko, ki, pairin, col50]
    g = g.reshape(8, 2, 2, 50, 4, 2, 128)  # (core, group, pairin, col50, ko, ki, p)
    g = np.transpose(g, (0, 6, 1, 4, 5, 2, 3))  # core, p, g, ko, ki, pairin, col50
    g = np.ascontiguousarray(g).reshape(8, 128, 2, 4, 2, 100)
    # pad col 100 -> 112: DoubleRow ldweights needs a 16B-aligned k-tile stride
    out = np.zeros((8, 128, 2, 4, 2, 112), np.float32)
    out[..., :100] = g
    return out.reshape(8, 128, 1792).astype(ml_dtypes.float8_e4m3)


def kernel(proj: np.ndarray) -> np.ndarray:
    from concourse.bass_utils import run_bass_kernel_spmd

    nc = get_nc()
    arr = pack_inputs(np.asarray(proj))
    in_maps = [{"u": arr[c]} for c in range(NCORES)]
    results = run_bass_kernel_spmd(nc, in_maps, list(range(NCORES))).results
    # device fin = [esum | -2*pos] per (partition-row, group); esum excludes
    # the +10 LSE shift: lse = ln(esum) + 10. loss = sum(lse - pos)/(2*R*B)
    total = 0.0
    for r in results:
        fin = np.asarray(r["out"], dtype=np.float64)
        total += float(np.sum(np.log(fin[:, 0:2])) + 10.0 * 200 + np.sum(fin[:, 2:4]))
    return np.float32(total / (2.0 * R * NB * NCORES))


# revision 13
# speedup vs baseline: 1.2421x; 1.0628x over previous
"""Trainium2 Bass kernel for CGL contrastive region loss.

Problem: proj (96, 256, 64, 64) f32 = 3 stacked views of B=32 images.
Only views 2 and 3 (aug1/aug2) are used. From each image, 25 regions
(5x5 grid of 2x2 windows at centres {10..50}) are extracted over all 256
channels -> region vectors of D = 256*2*2 = 1024. Per image pair the
loss reduces to: for each row r of the 50x50 Gram matrix G of the
stacked normalized regions [u1;u2] (scaled by 1/TAU), LSE over the full
row excluding only the main diagonal entry, minus the positive logit
pos_r = S[r, (r+25)%50]. Data-parallel over batch (4 pairs/core, 8
cores).

Device pipeline per core (v2):
  Host L2-normalizes each region vector and folds in sqrt(1/TAU), so the
  Gram IS the logit matrix S directly (diag exactly ~10). Input ships as
  fp8e4 [128, 1600]: free = (group:2, ko:4, ki:2, col:100), two pairs
  stacked per 100-col group.  One DMA on the sync ring.
  PE: per group, a bf16 [103,100]x[103,100] "mask matmul" seeds PSUM with
  M = -C*delta - C + C*sameblock (kills the main diagonal and the
  cross-pair garbage blocks; C=200), then 4 fp8 DoubleRow matmuls
  (K=256 each) accumulate the Gram on top -> PSUM holds S+M [100, 200].
  ACT: per group, exp(S+M-10) with accum_out -> row sums esum [100,2]
  directly (eall scratch write is dead).  DVE (parallel): -2*pos via a
  two-slice affine-select constant mul + blocked reduce -> [100,2].
  One 1600B output DMA ships fin=[esum|-2pos] [100,4]; host does
  ln(esum), the +10 LSE shift, and the global scale/sum.

ACT tables: only Exp is needed on device (ln runs on the host), served
by `exp_and_others`, forced single-set by pointing both bacc's
insert_act_table_loads and walrus (BASS_ACT_ROOT_JSON_PATH) at a patched
act_info.json in which no other set contains exp. The table load is
pulled to the head of the ACT queue by a dummy activation, hidden under
the input DMA.

Span overheads trimmed: Bass-init const memsets + entry all-engine
barrier deleted from the BIR (the NRT preamble already runs two
all-engine rendezvous and no const APs are referenced). Tile tail uses
a sem-only drain barrier. The NRT preamble (~7us to program start) and
postamble semaphore wipe (~7us) are runtime-injected and immovable.
"""

import os
import numpy as np

NB = 4                    # pairs per core
NCORES = 8
R = 25
_CENTRES = (10, 20, 30, 40, 50)
SQC = np.float32(np.sqrt(10.0))   # sqrt(1/TAU)
MASK_S = 200.0 ** 0.5             # sqrt(C): mask magnitude C=200

_nc_cache = None


def _patched_act_root():
    """Stage a copy of the neuronxcc pwp table dir whose act_info.json
    leaves `exp_and_others` as the only set containing exp, so the single
    activation function used on device resolves to one table set."""
    import json
    import shutil
    import tempfile

    import neuronxcc

    src = os.path.join(os.path.dirname(neuronxcc.__file__), "pwp", "pwp_bin_trainium")
    dst = os.path.join(tempfile.gettempdir(), "pwp_exponly_%d" % os.getuid())
    marker = os.path.join(dst, ".patched_ok")
    if not os.path.exists(marker):
        if os.path.exists(dst):
            shutil.rmtree(dst)
        shutil.copytree(src, dst)
        p = os.path.join(dst, "act_info.json")
        os.chmod(p, 0o644)
        with open(p) as f:
            d = json.load(f)
        for e in d["act_func_sets"]:
            if e["name"] != "exp_and_others":
                e["act"].pop("exp", None)
        with open(p, "w") as f:
            json.dump(d, f)
        with open(marker, "w") as f:
            f.write("ok")
    return os.path.join(dst, "act_info.json")


def _apply_act_surgery():
    import functools
    import json

    import concourse.bacc as baccmod

    act_json = _patched_act_root()
    os.environ["BASS_ACT_ROOT_JSON_PATH"] = act_json

    @functools.cache
    def patched_tables(arch):
        from concourse import mybir

        with open(act_json) as f:
            d = json.load(f)
        return {
            e["name"]: {
                mybir.ActivationFunctionType.from_pwp(v) for v in e["act"].keys()
            }
            for e in d["act_func_sets"]
        }

    baccmod.get_activation_tables = patched_tables


def _strip_init_overhead(nc):
    """Remove the Bass-init const memsets and entry all-engine barrier from
    the 'main' block. No const APs are referenced by this kernel, and the
    NRT preamble already synchronizes all engines before the program runs."""
    from concourse import mybir

    for func in nc.m.functions:
        for blk in func.blocks:
            if blk.name != "main":
                continue
            kept = []
            for inst in blk.instructions:
                if isinstance(
                    inst,
                    (mybir.InstMemset, mybir.InstDrain, mybir.InstEventSemaphore),
                ):
                    continue
                kept.append(inst)
            blk.instructions[:] = kept


def _build_nc():
    _apply_act_surgery()

    import concourse.bacc as bacc
    import concourse.tile as tile
    from concourse import mybir
    from concourse.vector_clock import ScopedClock

    class FastTailTileContext(tile.TileContext):
        """Tile tail without the two full all-engine barriers.

        The sync-engine drain already waits on the global vector clock
        (every instruction's sem tick), so once it completes nothing is
        in flight; a sem-only EVSEM barrier then orders the gpsimd
        sem_clears after it."""

        def _drain_and_barrier(self, tick_clock, wait_clock):
            drain_inst = self.nc.sync.drain()
            wait_clock.add_sem_waits(
                drain_inst.ins, ScopedClock({None: tick_clock.global_clock})
            )
            self.nc.all_engine_barrier(sem_only=True)
            popped = self.nc._tile_sem_poison_stack.pop()
            assert popped is self._sem_poison
            self.nc.clear_and_free_semaphores(list(self.sems.allocated().values()))

    f32 = mybir.dt.float32
    bf16 = mybir.dt.bfloat16
    fp8 = mybir.dt.float8e4
    Alu = mybir.AluOpType
    Act = mybir.ActivationFunctionType
    X = mybir.AxisListType.X
    DR = mybir.MatmulPerfMode.DoubleRow

    nc = bacc.Bacc("TRN2", target_bir_lowering=False, debug=False)
    u_dram = nc.dram_tensor("u", [128, 1792], fp8, kind="ExternalInput").ap()
    out_dram = nc.dram_tensor("out", [1, 400], f32, kind="ExternalOutput").ap()

    with FastTailTileContext(nc) as tc:
        with (
            tc.tile_pool(name="data", bufs=1) as data,
            tc.tile_pool(name="consts", bufs=1) as consts,
            tc.tile_pool(name="work", bufs=2) as work,
            tc.tile_pool(name="psg0", bufs=1, space="PSUM") as psg0,
            tc.tile_pool(name="psg1", bufs=1, space="PSUM") as psg1,
            tc.tile_pool(name="pst", bufs=1, space="PSUM") as pst,
            tc.tile_pool(name="pso", bufs=1, space="PSUM") as pso,
        ):
            # input DMA first: one [128, 1792] fp8 transfer on the sync ring
            ub = data.tile([128, 1792], fp8, tag="ub")
            nc.sync.dma_start(ub[:], u_dram)

            # ---- on-device constants (synthesized during the DMA window) ----
            # All compute-engine APs must start at partition 0, so the mask
            # matmul operands live in three aligned tiles -> 3 tiny matmuls:
            #   diag [100,100]: stat -s / mov +s on the diagonal -> -C*delta
            #   crow [1,100]:   stat -s / mov +s everywhere      -> -C
            #   brow [2,100]:   both +s on 50-block indicators   -> +C*same
            dstat = consts.tile([100, 100], bf16, tag="dstat")
            dmov = consts.tile([100, 100], bf16, tag="dmov")
            nc.vector.memset(dstat[:], 0.0)
            nc.vector.memset(dmov[:], 0.0)
            # (affine_select runs on gpsimd; idle during the DMA window)
            nc.gpsimd.affine_select(
                dstat[:], dstat[:],
                pattern=[[1, 100]], compare_op=Alu.not_equal,
                fill=-MASK_S, base=0, channel_multiplier=-1,
            )
            nc.gpsimd.affine_select(
                dmov[:], dmov[:],
                pattern=[[1, 100]], compare_op=Alu.not_equal,
                fill=MASK_S, base=0, channel_multiplier=-1,
            )
            cstat = consts.tile([1, 100], bf16, tag="cstat")
            cmov = consts.tile([1, 100], bf16, tag="cmov")
            nc.vector.memset(cstat[:], -MASK_S)
            nc.vector.memset(cmov[:], MASK_S)
            # brow row p covers cols [50p, 50p+50)
            brow = consts.tile([2, 100], bf16, tag="brow")
            nc.vector.memset(brow[:], MASK_S)
            nc.gpsimd.affine_select(
                brow[:], brow[:],
                pattern=[[1, 100]], compare_op=Alu.is_ge,
                fill=0.0, base=0, channel_multiplier=-50,
            )
            nc.gpsimd.affine_select(
                brow[:], brow[:],
                pattern=[[-1, 100]], compare_op=Alu.is_gt,
                fill=0.0, base=50, channel_multiplier=50,
            )

            # negident: -2 at (r, 25+r) for r in [0,25) and (50+r, 75+r).
            # op1 marks the full f-p==25 stripe (also hits rows 25..49 at
            # f in [50,75), which land on garbage cross-blocks); op2 zeroes
            # the f in [50,75) band, which contains no wanted entries.
            negid = consts.tile([100, 100], f32, tag="negid")
            nc.vector.memset(negid[:], 0.0)
            nc.gpsimd.affine_select(
                negid[:], negid[:],
                pattern=[[1, 100]], compare_op=Alu.not_equal,
                fill=-2.0, base=-25, channel_multiplier=-1,
            )
            nc.gpsimd.affine_select(
                negid[:], negid[:],
                pattern=[[-25, 2], [1, 50]], compare_op=Alu.is_ge,
                fill=0.0, base=0, channel_multiplier=0,
            )

            # bias column for exp(S - 10); ones column for the sum matmuls
            b_m10 = consts.tile([100, 1], f32, tag="bm10")
            nc.vector.memset(b_m10[:], -10.0)
            onesb = consts.tile([100, 1], bf16, tag="onesb")
            nc.vector.memset(onesb[:], 1.0)

            # dummy activation on a memset scratch (no DMA deps): pulls the
            # single ACT table load to the head of the ACT queue, fully
            # hidden under the input DMA
            tscr = work.tile([1, 1], f32, tag="tscr")
            nc.vector.memset(tscr[:], 1.0)
            nc.scalar.activation(tscr[:], tscr[:], Act.Exp, bias=tscr[:])

            # PE p-state warmup: sustained dummy matmuls during the input
            # DMA window so the real gram chains run at full clock
            wscr = work.tile([128, 400], bf16, tag="wscr")
            nc.vector.memset(wscr[:], 0.0)
            psw = pst.tile([50, 400], f32, tag="warm")
            for w in range(3):
                nc.tensor.matmul(
                    psw[:], wscr[:, 0:50], wscr[:],
                    start=(w == 0), stop=(w == 2),
                )

            # ---- gram + mask: separate PSUM banks per group ----
            gp0 = psg0.tile([100, 100], f32, tag="g0")
            gp1 = psg1.tile([100, 100], f32, tag="g1")
            gps = [gp0, gp1]
            # mask matmuls first (consts only: run fully under the DMA)
            for g in range(2):
                gs = gps[g][:]
                nc.tensor.matmul(gs, dstat[:], dmov[:], start=True, stop=False,
                                 skip_group_check=True)
                nc.tensor.matmul(gs, cstat[:], cmov[:], start=False, stop=False,
                                 skip_group_check=True)
                nc.tensor.matmul(gs, brow[:], brow[:], start=False, stop=False,
                                 skip_group_check=True)
            # fp8 DoubleRow gram chains (K=256 each)
            for g in range(2):
                gs = gps[g][:]
                for ko in range(4):
                    # col dim padded 100->112: DoubleRow needs the k-tile
                    # stride 16B-aligned; only cols 0:100 are read
                    sl = ub[:, g * 896 + ko * 224 : g * 896 + (ko + 1) * 224]
                    sl = sl.rearrange("p (ki c) -> p ki c", ki=2)[:, :, 0:100]
                    nc.tensor.matmul(gs, sl, sl, start=False, stop=(ko == 3),
                                     perf_mode=DR, skip_group_check=True)

            # exp(S + M - 10) -> SBUF bf16; the Gram is symmetric, so row
            # sums == column sums and the esum/pos reductions collapse to
            # ones^T @ [eall | pmul] matmuls with a [1, 400] PSUM result.
            eact = work.tile([100, 200], bf16, tag="eact")
            for g in range(2):
                nc.scalar.activation(
                    eact[:, g * 100 : (g + 1) * 100], gps[g][:],
                    Act.Exp, bias=b_m10,
                )
            edve = work.tile([100, 200], bf16, tag="edve")
            for g in range(2):
                nc.vector.tensor_mul(
                    edve[:, g * 100 : (g + 1) * 100], gps[g][:], negid[:],
                )

            po = pso.tile([1, 400], f32, tag="po")
            nc.tensor.matmul(po[:, 0:200], onesb[:], eact[:], start=True, stop=True)
            nc.tensor.matmul(po[:, 200:400], onesb[:], edve[:], start=True, stop=True)
            fot = work.tile([1, 400], f32, tag="fot")
            nc.vector.tensor_copy(fot[:], po[:])

            # single-descriptor output DMA; ln + shift + scaling on the host
            nc.sync.dma_start(out_dram, fot[:])

    _strip_init_overhead(nc)
    nc.compile()
    return nc


def get_nc():
    global _nc_cache
    if _nc_cache is None:
        _nc_cache = _build_nc()
    return _nc_cache


def pack_inputs(proj: np.ndarray) -> np.ndarray:
    """(96,256,64,64) -> (8, 128, 1600) fp8e4: per core, partition=feature
    p (f = (ko*2+ki)*128 + p), free=(group, ko, ki, pairin*50 + view*25+reg).
    Region vectors are L2-normalized and scaled by sqrt(1/TAU) on the host,
    so the device Gram is the logit matrix directly."""
    import ml_dtypes

    win = np.array([[c - 1, c] for c in _CENTRES])  # (5, 2)
    v = np.stack([proj[32:64], proj[64:96]], axis=1)  # (32, 2, 256, 64, 64)
    g = v[:, :, :, win[:, :, None, None], win[None, None, :, :]]  # (32,2,256,5,2,5,2)
    # region vector = flatten (C, dy, dx); reorder to (b, view, rh, rw, C, dy, dx)
    g = np.transpose(g, (0, 1, 3, 5, 2, 4, 6)).reshape(32, 2, 25, 1024)
    nrm = np.sqrt(np.sum(g.astype(np.float32) ** 2, axis=-1, keepdims=True))
    g = g / np.maximum(nrm, 1e-12) * SQC  # (32, 2, 25, 1024)
    # stack views: col50 = view*25 + reg
    g = g.reshape(32, 50, 1024)
    # feature f -> (ko, ki, p)
    g = g.reshape(32, 50, 4, 2, 128)
    # per core: [pair(4), col50, ko, ki, p] -> [p, group, ko, ki, pairin, col50]
    g = g.reshape(8, 2, 2, 50, 4, 2, 128)  # (core, group, pairin, col50, ko, ki, p)
    g = np.transpose(g, (0, 6, 1, 4, 5, 2, 3))  # core, p, g, ko, ki, pairin, col50
    g = np.ascontiguousarray(g).reshape(8, 128, 2, 4, 2, 100)
    # pad col 100 -> 112: DoubleRow ldweights needs a 16B-aligned k-tile stride
    out = np.zeros((8, 128, 2, 4, 2, 112), np.float32)
    out[..., :100] = g
    return out.reshape(8, 128, 1792).astype(ml_dtypes.float8_e4m3)


def kernel(proj: np.ndarray) -> np.ndarray:
    from concourse.bass_utils import run_bass_kernel_spmd

    nc = get_nc()
    arr = pack_inputs(np.asarray(proj))
    in_maps = [{"u": arr[c]} for c in range(NCORES)]
    results = run_bass_kernel_spmd(nc, in_maps, list(range(NCORES))).results
    # device out = [esum cols (200) | -2*pos cols (200)]; esum excludes the
    # +10 LSE shift: lse = ln(esum) + 10. loss = sum(lse - pos)/(2*R*B)
    total = 0.0
    for r in results:
        fin = np.asarray(r["out"], dtype=np.float64).reshape(400)
        total += float(np.sum(np.log(fin[0:200])) + 10.0 * 200 + np.sum(fin[200:400]))
    return np.float32(total / (2.0 * R * NB * NCORES))
